# revision 29
# baseline (speedup 1.0000x reference)
"""DKVMN forward kernel for 8 Trainium2 NeuronCores (Bass/Tile).

Self-contained: takes the full un-sharded inputs of the reference
`setup_inputs()`, shards batch-parallel across 8 cores, runs one Bass/Tile
program per core, and assembles the reference's 4 outputs.

Per-core structure (8 batch elements each, T=512 steps):
  A: embedding gathers (indirect DMA) -> feature-major transposes (PE with
     diagonal 1/c_num rhs) -> k/v/e/a/w heads, packed into *_all tiles.
  B: DKVMN recurrence. Memory state flattened (m,d) -> 25 chunks of 128
     partitions; time on the free axis. Per chunk and per pair of batch
     elements: replicate w rows (PE), alpha = 1 - w*e (DVE+ACT),
     beta = w*a (GPSIMD), hardware scan (tensor_tensor_scan), read values
     via w*u (DVE/GPSIMD) reduced on PE into persistent PSUM accumulators.
  C: f/zsum/q heads per batch element + PE transposes to token-major.
  D: 2-class losses and pred via sigmoid/ln identities, masked reduce.
"""
import sys

sys.path.insert(0, '/opt/trn_rl_repo')

from contextlib import ExitStack

import numpy as np

import concourse.bacc as bacc
import concourse.bass as bass
import concourse.tile as tile
from concourse import mybir
from concourse.bass_utils import run_bass_kernel_spmd
from concourse.masks import make_identity

F = mybir.dt.float32
I32 = mybir.dt.int32
AF = mybir.ActivationFunctionType
OP = mybir.AluOpType

B, L, K, D, M = 64, 511, 4, 64, 50
T = L + 1            # 512
NCORE = 8
BL = B // NCORE      # 8 batch elements per core
NCHUNK = (M * D) // 128   # 25
EPS = 1e-12

# per 2-batch group: how many of the 2 tmat multiplies / scans run on GPSIMD
TMAT_GP = 1
SCAN_GP = 0
PHASES = 'ABCD'
CSUB = 4


def host_prep(inputs):
    f32 = np.float32
    qseqs = np.asarray(inputs['qseqs']); cseqs = np.asarray(inputs['cseqs'])
    rseqs = np.asarray(inputs['rseqs'])
    sq = np.asarray(inputs['shft_qseqs']); sc = np.asarray(inputs['shft_cseqs'])
    sr = np.asarray(inputs['shft_rseqs'])
    cq = np.concatenate([qseqs[:, :1], sq], 1).astype(np.int64)
    cc = np.concatenate([cseqs[:, :1, :], sc], 1).astype(np.int64)
    cr = np.concatenate([rseqs[:, :1], sr], 1)
    c_num = np.maximum((cc >= 0).sum(-1), 1)
    inv_c = (1.0 / c_num).astype(f32)
    r = cr.astype(f32)

    Wk = np.asarray(inputs['Wk'], f32); bk = np.asarray(inputs['bk'], f32)
    Wv = np.asarray(inputs['Wv'], f32); bv = np.asarray(inputs['bv'], f32)
    Mk = np.asarray(inputs['Mk'], f32); Mv0 = np.asarray(inputs['Mv0'], f32)
    We = np.asarray(inputs['We'], f32); be = np.asarray(inputs['be'], f32)
    Wa = np.asarray(inputs['Wa'], f32); ba = np.asarray(inputs['ba'], f32)
    Wf = np.asarray(inputs['Wf'], f32); bf = np.asarray(inputs['bf'], f32)
    Wp = np.asarray(inputs['Wp'], f32); bp = np.asarray(inputs['bp'], f32)
    Wq = np.asarray(inputs['Wq'], f32); bq = np.asarray(inputs['bq'], f32)
    Ws = np.asarray(inputs['Ws'], f32); bs = np.asarray(inputs['bs'], f32)
    c_tab = np.ascontiguousarray(np.asarray(inputs['c_emb_table'], f32))
    q_tab = np.ascontiguousarray(np.asarray(inputs['q_emb_table'], f32))
    const = np.float32(np.asarray(inputs['constant']))
    NQ = q_tab.shape[0]

    SelW = np.zeros((M, NCHUNK * 128), f32)
    for ci in range(NCHUNK):
        for half in range(2):
            SelW[2 * ci + half,
                 ci * 128 + half * 64: ci * 128 + (half + 1) * 64] = 1.0
    E64 = np.zeros((64, 128), f32)
    for p in range(128):
        E64[p % 64, p] = 1.0

    Wf_top = np.ascontiguousarray(Wf[:D])
    shared = dict(
        q_tab=q_tab, c_tab=c_tab,
        Wk=Wk, Wv_bot=np.ascontiguousarray(Wv[2 * D:]),
        Wv_dif=np.ascontiguousarray(Wv[:2 * D] - Wv[2 * D:]).astype(f32),
        MkT=np.ascontiguousarray(Mk.T), We=We, Wa=Wa,
        Wf_top2=np.vstack([Wf_top, Wf_top]).astype(f32),
        Wf_bot=np.ascontiguousarray(Wf[D:]),
        lhsT_p=np.vstack([Wp, (bp + bq + bs)[None]]).astype(f32),
        Ws2=np.vstack([Ws, Ws]).astype(f32), Wq_h=Wq,
        lhsT_qq=np.vstack([Wq, (bq + 2.0 * const)[None]]).astype(f32),
        biases=np.stack([bk, bv, be, ba, bf], 1).astype(f32),
        Mv0c=np.ascontiguousarray(Mv0.reshape(M * D).reshape(NCHUNK, 128).T),
        SelW=SelW, E64=E64,
        E64T=np.ascontiguousarray(E64.T),
        ones128=np.ones((128, 1), f32),
        ones1=np.ones((1, 128), f32),
    )

    per_core = []
    for c in range(NCORE):
        bsl = slice(c * BL, (c + 1) * BL)
        cq_c, cc_c = cq[bsl], cc[bsl]
        sr_c = np.asarray(sr[bsl])
        idx_q = np.zeros((BL, 128, 4), np.int32)
        idx_c = np.zeros((BL, 128, 16), np.int32)
        for b in range(BL):
            for j in range(4):
                toks = np.arange(128) + 128 * j
                idx_q[b, :, j] = cq_c[b, toks]
                for k in range(K):
                    idx_c[b, :, k * 4 + j] = cc_c[b, toks, k] + 1
        s_all = np.zeros((128, 32), f32)
        mask = np.zeros((128, 32), f32)
        invc_col = np.zeros((BL, 128, 4), f32)
        for b in range(BL):
            for j in range(4):
                toks = np.arange(128) + 128 * j
                valid = toks >= 1
                lab = np.zeros(128, np.int64)
                lab[valid] = sr_c[b, toks[valid] - 1]
                s_all[:, b * 4 + j] = 2.0 * lab - 1.0
                mask[:, b * 4 + j] = valid.astype(f32)
                invc_col[b, :, j] = inv_c[c * BL + b, toks]
        per_core.append(dict(
            idx_q=idx_q, idx_c=idx_c,
            r_row=r[bsl].astype(f32), invc_col=invc_col,
            s_all=s_all, mask_all=mask,
        ))

    t_out = sr.reshape(-1).astype(np.int32)
    qf_out = (np.asarray(sq).reshape(-1).astype(np.int64)
              + NQ * sr.reshape(-1).astype(np.int64)).astype(np.int32)
    return shared, per_core, t_out, qf_out


SHARED_SPECS = [
    ('q_tab', None, F), ('c_tab', None, F),
    ('Wk', [2 * D, D], F), ('Wv_bot', [2 * D, D], F), ('Wv_dif', [2 * D, D], F),
    ('MkT', [D, M], F), ('We', [D, D], F), ('Wa', [D, D], F),
    ('Wf_top2', [128, D], F), ('Wf_bot', [D, D], F),
    ('lhsT_p', [D + 1, 2], F), ('Ws2', [128, 2], F), ('Wq_h', [D, 2], F),
    ('lhsT_qq', [D + 1, 2], F),
    ('biases', [D, 5], F), ('Mv0c', [128, NCHUNK], F),
    ('SelW', [M, NCHUNK * 128], F), ('E64', [D, 128], F), ('E64T', [128, D], F),
    ('ones128', [128, 1], F), ('ones1', [1, 128], F),
]
PER_CORE_SPECS = [
    ('idx_q', [BL, 128, 4], I32), ('idx_c', [BL, 128, 16], I32),
    ('r_row', [BL, T], F), ('invc_col', [BL, 128, 4], F),
    ('s_all', [128, 32], F), ('mask_all', [128, 32], F),
]


def _dummy_outputs(nc, sd, dram):
    p_ = sd.tile([128, BL * 4], F, name='dummy_pred')
    nc.gpsimd.memset(p_[:], 0.0)
    nc.sync.dma_start(dram['pred_o'][:], p_[:])
    l_ = sd.tile([1, 1], F, name='dummy_loss')
    nc.gpsimd.memset(l_[:], 0.0)
    nc.sync.dma_start(dram['loss_o'][:], l_[:])


def build_program(ctx: ExitStack, tc: tile.TileContext, dram, reps=1):
    nc = tc.nc

    consts = ctx.enter_context(tc.tile_pool(name='consts', bufs=1))
    sbA = ctx.enter_context(tc.tile_pool(name='sbA', bufs=2))
    sbAll = ctx.enter_context(tc.tile_pool(name='sbAll', bufs=1))
    sbA1 = ctx.enter_context(tc.tile_pool(name='sbA1', bufs=1))
    sbB = ctx.enter_context(tc.tile_pool(name='sbB', bufs=2))
    sbD = ctx.enter_context(tc.tile_pool(name='sbD', bufs=1))
    # one shared PSUM pool for transients ([128,1024]-sized slots = 2 banks
    # x 2 bufs = 4 banks; everything uses tag 'big') + 4 persistent
    # read-accumulator banks (bufs=1, four tags)
    psB = ctx.enter_context(tc.tile_pool(name='psB', bufs=2, space='PSUM'))
    psR = ctx.enter_context(tc.tile_pool(name='psR', bufs=1, space='PSUM'))

    # ---- constants / weights
    cs = {}
    for name, shape, dt in SHARED_SPECS:
        if name in ('q_tab', 'c_tab'):
            continue
        t_ = consts.tile(shape, dt, tag=f'c_{name}')
        nc.sync.dma_start(t_[:], dram[name][:])
        cs[name] = t_
    for name, shape, dt in PER_CORE_SPECS:
        if name in ('idx_q', 'idx_c', 'r_row', 'invc_col'):
            continue
        t_ = consts.tile(shape, dt, tag=f'c_{name}')
        nc.sync.dma_start(t_[:], dram[name][:])
        cs[name] = t_
    eps_t = consts.tile([128, 1], F)
    nc.gpsimd.memset(eps_t[:], EPS)
    ident2 = consts.tile([2, 2], F)
    make_identity(nc, ident2[:])
    ident128 = consts.tile([128, 128], F)
    make_identity(nc, ident128[:])
    zrow = consts.tile([1, 128], F)
    nc.gpsimd.memset(zrow[:], 0.0)
    r_tiles, ic_tiles = [], []
    for b in range(BL):
        rt = consts.tile([1, T], F, tag=f'r_{b}')
        nc.sync.dma_start(rt[:], dram['r_row'][b:b + 1, :])
        r_tiles.append(rt)
        it = consts.tile([128, 4], F, tag=f'ic_{b}')
        nc.sync.dma_start(it[:], dram['invc_col'][b, :, :])
        ic_tiles.append(it)

    # persistent packed activations (columns = b*T + t)
    kT_all = sbAll.tile([D + 1, BL * T], F)
    nc.gpsimd.memset(kT_all[D:D + 1, :], 1.0)
    wT_all = sbAll.tile([M, BL * T], F)
    erep_all = sbAll.tile([128, BL * T], F)
    arep_all = sbAll.tile([128, BL * T], F)
    rvT_all = sbAll.tile([D, BL * T], F)

    rep_ctx = tc.For_i(0, reps, 1) if reps > 1 else None
    if rep_ctx is not None:
        ctx.enter_context(rep_ctx)

    # ================= phase A =================
    for b in range(BL):
        bs_ = slice(b * T, (b + 1) * T)
        idxq_b = sbA.tile([128, 4], I32, tag='idxq')
        nc.sync.dma_start(idxq_b[:], dram['idx_q'][b, :, :])
        idxc_b = sbA.tile([128, 16], I32, tag='idxc')
        nc.sync.dma_start(idxc_b[:], dram['idx_c'][b, :, :])
        qg = sbA.tile([128, 4 * D], F, tag='qg')
        for j in range(4):
            nc.gpsimd.indirect_dma_start(
                out=qg[:, j * D:(j + 1) * D], out_offset=None,
                in_=dram['q_tab'][:],
                in_offset=bass.IndirectOffsetOnAxis(
                    ap=idxq_b[:, j:j + 1], axis=0))
        cg = sbA1.tile([128, 16 * D], F, tag='cg')
        for kj in range(16):
            nc.gpsimd.indirect_dma_start(
                out=cg[:, kj * D:(kj + 1) * D], out_offset=None,
                in_=dram['c_tab'][:],
                in_offset=bass.IndirectOffsetOnAxis(
                    ap=idxc_b[:, kj:kj + 1], axis=0))

        diag = sbA1.tile([128, 4 * 128], F, tag='diag')
        for j in range(4):
            nc.gpsimd.tensor_scalar_mul(diag[:, j * 128:(j + 1) * 128],
                                        ident128[:], ic_tiles[b][:, j:j + 1])

        x_ps = psB.tile([128, T], F, tag='big')
        for j in range(4):
            nc.tensor.matmul(x_ps[0:D, 128 * j:128 * (j + 1)],
                             qg[:, j * D:(j + 1) * D],
                             ident128[:], start=True, stop=True)
            for k in range(K):
                nc.tensor.matmul(x_ps[D:2 * D, 128 * j:128 * (j + 1)],
                                 cg[:, (k * 4 + j) * D:(k * 4 + j + 1) * D],
                                 diag[:, j * 128:(j + 1) * 128],
                                 start=(k == 0), stop=(k == K - 1))
        x = sbA.tile([128, T], F, tag='x')
        nc.scalar.copy(x[:], x_ps[:])

        rr_ps = psB.tile([128, T], F, tag='big')
        nc.tensor.matmul(rr_ps[:], cs['ones1'][:], r_tiles[b][:],
                         start=True, stop=True)
        x_r = sbA.tile([128, T], F, tag='x_r')
        nc.vector.tensor_mul(x_r[:], x[:], rr_ps[:])

        k_ps = psB.tile([D, T], F, tag='big')
        nc.tensor.matmul(k_ps[:], cs['Wk'][:], x[:], start=True, stop=True)
        nc.scalar.activation(kT_all[0:D, bs_], k_ps[:], AF.Identity,
                             bias=cs['biases'][:, 0:1], scale=1.0)

        v_ps = psB.tile([D, T], F, tag='big')
        nc.tensor.matmul(v_ps[:], cs['Wv_bot'][:], x[:], start=True, stop=False)
        nc.tensor.matmul(v_ps[:], cs['Wv_dif'][:], x_r[:], start=False, stop=True)
        vT = sbA1.tile([D, T], F, tag='vT')
        nc.scalar.activation(vT[:], v_ps[:], AF.Identity,
                             bias=cs['biases'][:, 1:2], scale=1.0)

        e_ps = psB.tile([D, T], F, tag='big')
        nc.tensor.matmul(e_ps[:], cs['We'][:], vT[:], start=True, stop=True)
        eT = sbA1.tile([D, T], F, tag='eT')
        nc.scalar.activation(eT[:], e_ps[:], AF.Sigmoid,
                             bias=cs['biases'][:, 2:3], scale=1.0)
        a_ps = psB.tile([D, T], F, tag='big')
        nc.tensor.matmul(a_ps[:], cs['Wa'][:], vT[:], start=True, stop=True)
        aT = sbA1.tile([D, T], F, tag='aT')
        nc.scalar.activation(aT[:], a_ps[:], AF.Tanh,
                             bias=cs['biases'][:, 3:4], scale=1.0)

        wl_ps = psB.tile([M, T], F, tag='big')
        nc.tensor.matmul(wl_ps[:], cs['MkT'][:], kT_all[0:D, bs_],
                         start=True, stop=True)
        expw = sbA1.tile([M, T], F, tag='expw')
        nc.scalar.activation(expw[:], wl_ps[:], AF.Exp)
        sums_ps = psB.tile([1, T], F, tag='big')
        nc.tensor.matmul(sums_ps[:], cs['ones128'][0:M, :], expw[:],
                         start=True, stop=True)
        lse = sbA.tile([1, T], F, tag='lse')
        nc.scalar.activation(lse[:], sums_ps[:], AF.Ln)
        einv = sbA.tile([1, T], F, tag='einv')
        nc.scalar.activation(einv[:], lse[:], AF.Exp, bias=0.0, scale=-1.0)
        er_ps = psB.tile([M, T], F, tag='big')
        nc.tensor.matmul(er_ps[:], cs['ones1'][:, 0:M], einv[:],
                         start=True, stop=True)
        nc.vector.tensor_mul(wT_all[:, bs_], expw[:], er_ps[:])

        erep_ps = psB.tile([128, T], F, tag='big')
        nc.tensor.matmul(erep_ps[:], cs['E64'][:], eT[:], start=True, stop=True)
        nc.scalar.copy(erep_all[:, bs_], erep_ps[:])
        arep_ps = psB.tile([128, T], F, tag='big')
        nc.tensor.matmul(arep_ps[:], cs['E64'][:], aT[:], start=True, stop=True)
        nc.scalar.copy(arep_all[:, bs_], arep_ps[:])

    if 'B' not in PHASES:
        _dummy_outputs(nc, sbD, dram)
        return
    # ================= phase B =================
    read_tiles = [psR.tile([128, T], F, tag=f'read{g}', name=f'read{g}')
                  for g in range(BL // 2)]
    for ci in range(NCHUNK):
        sel = cs['SelW'][:, ci * 128:(ci + 1) * 128]
        mv0 = cs['Mv0c'][:, ci:ci + 1]
        for g in range(BL // 2):
            b0 = 2 * g
            gsl = slice(b0 * T, (b0 + 2) * T)
            wr_ps = psB.tile([128, 2 * T], F, tag='big')
            nc.tensor.matmul(wr_ps[:, 0:T], sel, wT_all[:, b0 * T:(b0 + 1) * T],
                             start=True, stop=True)
            nc.tensor.matmul(wr_ps[:, T:2 * T], sel,
                             wT_all[:, (b0 + 1) * T:(b0 + 2) * T],
                             start=True, stop=True)
            we2 = sbB.tile([128, 2 * T], F, tag='we')
            nc.vector.tensor_mul(we2[:], wr_ps[:], erep_all[:, gsl])
            alpha2 = sbB.tile([128, 2 * T], F, tag='alpha')
            nc.scalar.activation(alpha2[:], we2[:], AF.Copy, bias=1.0, scale=-1.0)
            wrs2 = sbB.tile([128, 2 * T], F, tag='wrs')
            nc.scalar.copy(wrs2[:], wr_ps[:])
            beta2 = sbB.tile([128, 2 * T], F, tag='beta')
            nc.gpsimd.tensor_mul(beta2[:], wrs2[:], arep_all[:, gsl])
            u2 = sbB.tile([128, 2 * (T + 1)], F, tag='u')
            tmat2 = sbA1.tile([128, 2 * T], F, tag='tmat')
            for i in range(2):
                nc.scalar.copy(u2[:, i * (T + 1):i * (T + 1) + 1], mv0)
                seng = nc.gpsimd if i < SCAN_GP else nc.vector
                seng.tensor_tensor_scan(
                    u2[:, i * (T + 1) + 1:(i + 1) * (T + 1)],
                    alpha2[:, i * T:(i + 1) * T],
                    beta2[:, i * T:(i + 1) * T],
                    mv0, OP.mult, OP.add)
                if i < TMAT_GP:
                    nc.gpsimd.tensor_mul(tmat2[:, i * T:(i + 1) * T],
                                         wrs2[:, i * T:(i + 1) * T],
                                         u2[:, i * (T + 1):i * (T + 1) + T])
                else:
                    nc.vector.tensor_mul(tmat2[:, i * T:(i + 1) * T],
                                         wr_ps[:, i * T:(i + 1) * T],
                                         u2[:, i * (T + 1):i * (T + 1) + T])
                nc.tensor.matmul(
                    read_tiles[g][64 * i:64 * i + D, :],
                    cs['E64T'][:], tmat2[:, i * T:(i + 1) * T],
                    start=(ci == 0), stop=(ci == NCHUNK - 1),
                    skip_group_check=True)

    if 'C' not in PHASES:
        # close read groups by consuming them trivially
        for g in range(BL // 2):
            sink = sbD.tile([128, T], F, tag=f'sink{g}', name=f'sink{g}')
            nc.scalar.copy(sink[:], read_tiles[g][:])
        _dummy_outputs(nc, sbD, dram)
        return
    # ================= phase C =================
    zT_sb = sbD.tile([128, BL * 8], F)
    qT_sb = sbD.tile([128, BL * 8], F)
    for g in range(BL // 2):
        nc.scalar.copy(rvT_all[:, (2 * g) * T:(2 * g + 1) * T],
                       read_tiles[g][0:D, :])
        nc.scalar.copy(rvT_all[:, (2 * g + 1) * T:(2 * g + 2) * T],
                       read_tiles[g][D:2 * D, :])
    if CSUB < 2:
        _dummy_outputs(nc, sbD, dram)
        return
    for b in range(BL):
        bs_ = slice(b * T, (b + 1) * T)
        rv_slice = rvT_all[:, bs_]
        f_ps = psB.tile([D, T], F, tag='big')
        nc.tensor.matmul(f_ps[:], cs['Wf_top2'][0:D, :], rv_slice,
                         start=True, stop=False)
        nc.tensor.matmul(f_ps[:], cs['Wf_bot'][:], kT_all[0:D, bs_],
                         start=False, stop=True)
        fT = sbA.tile([D + 1, T], F, tag='fT')
        nc.gpsimd.memset(fT[D:D + 1, :], 1.0)
        nc.scalar.activation(fT[0:D, :], f_ps[:], AF.Tanh,
                             bias=cs['biases'][:, 4:5], scale=1.0)
        if CSUB < 3:
            continue
        zs_ps = psB.tile([2, T], F, tag='big')
        nc.tensor.matmul(zs_ps[:], cs['lhsT_p'][:], fT[:], start=True, stop=False)
        nc.tensor.matmul(zs_ps[:], cs['Ws2'][0:D, :], rv_slice,
                         start=False, stop=False)
        nc.tensor.matmul(zs_ps[:], cs['Wq_h'][:], kT_all[0:D, bs_],
                         start=False, stop=True)
        zsum_sb = sbA.tile([2, T], F, tag='zsum')
        nc.scalar.copy(zsum_sb[:], zs_ps[:])
        qh_ps = psB.tile([2, T], F, tag='big')
        nc.tensor.matmul(qh_ps[:], cs['lhsT_qq'][:], kT_all[:, bs_],
                         start=True, stop=True)
        qh_sb = sbA.tile([2, T], F, tag='qh')
        nc.scalar.copy(qh_sb[:], qh_ps[:])
        if CSUB < 4:
            continue
        tp_ps = psB.tile([128, 16], F, tag='big')
        for j in range(4):
            nc.tensor.matmul(tp_ps[:, 2 * j:2 * j + 2],
                             zsum_sb[:, 128 * j:128 * (j + 1)],
                             ident2[:], start=True, stop=True)
            nc.tensor.matmul(tp_ps[:, 8 + 2 * j:8 + 2 * j + 2],
                             qh_sb[:, 128 * j:128 * (j + 1)],
                             ident2[:], start=True, stop=True)
        nc.scalar.copy(zT_sb[:, b * 8:(b + 1) * 8], tp_ps[:, 0:8])
        nc.scalar.copy(qT_sb[:, b * 8:(b + 1) * 8], tp_ps[:, 8:16])

    if 'D' not in PHASES or CSUB < 4:
        _dummy_outputs(nc, sbD, dram)
        return
    # ================= phase D =================
    NT = BL * 4
    sd = sbD
    qraw = qT_sb
    sigz = sd.tile([128, 2 * NT], F)
    nc.scalar.activation(sigz[:], zT_sb[:], AF.Sigmoid)
    sigq = sd.tile([128, 2 * NT], F)
    nc.scalar.activation(sigq[:], qT_sb[:], AF.Sigmoid)
    z_qks = sd.tile([128, 2 * NT], F)
    nc.scalar.activation(z_qks[:], sigz[:], AF.Ln, bias=eps_t[:], scale=1.0)
    z_q = sd.tile([128, 2 * NT], F)
    nc.scalar.activation(z_q[:], sigq[:], AF.Ln, bias=eps_t[:], scale=1.0)

    def coldiff(name, src):
        t_ = sd.tile([128, NT], F, tag=name)
        nc.vector.tensor_sub(t_[:], src[:, 1::2], src[:, 0::2])
        return t_

    d_qks = coldiff('d_qks', z_qks)
    d_q = coldiff('d_q', z_q)
    d_qraw = coldiff('d_qraw', qraw)
    d_core = sd.tile([128, NT], F)
    nc.vector.tensor_sub(d_core[:], d_qks[:], d_q[:])
    pred_sb = sd.tile([128, NT], F)
    nc.scalar.activation(pred_sb[:], d_core[:], AF.Sigmoid)
    nc.sync.dma_start(dram['pred_o'][:], pred_sb[:])

    sigd = sd.tile([128, NT], F)
    nc.scalar.activation(sigd[:], d_qks[:], AF.Sigmoid)
    sdqr = sd.tile([128, NT], F)
    nc.vector.tensor_mul(sdqr[:], d_qraw[:], cs['s_all'][:])
    sdqk = sd.tile([128, NT], F)
    nc.vector.tensor_mul(sdqk[:], d_qks[:], cs['s_all'][:])

    def softplus(name, src, scale):
        e_ = sd.tile([128, NT], F, tag=name + '_e')
        nc.scalar.activation(e_[:], src[:], AF.Exp, bias=0.0, scale=scale)
        s_ = sd.tile([128, NT], F, tag=name + '_s')
        nc.scalar.activation(s_[:], e_[:], AF.Ln, bias=1.0, scale=1.0)
        return s_

    sp1 = softplus('sp1', sdqr, -1.0)
    sp2 = softplus('sp2', sdqk, -1.0)
    sp3 = softplus('sp3', d_q, 1.0)
    t4 = sd.tile([128, NT], F)
    nc.vector.tensor_mul(t4[:], sigd[:], d_q[:])
    u1 = sd.tile([128, NT], F)
    nc.vector.tensor_add(u1[:], sp1[:], sp2[:])
    u2d = sd.tile([128, NT], F)
    nc.vector.tensor_sub(u2d[:], sp3[:], t4[:])
    tot = sd.tile([128, NT], F)
    nc.vector.tensor_add(tot[:], u1[:], u2d[:])
    scr = sd.tile([128, NT], F)
    acc = sd.tile([128, 1], F)
    nc.vector.scalar_tensor_tensor(scr[:], tot[:], 1.0, cs['mask_all'][:],
                                   OP.bypass, OP.mult, accum_out=acc[:])
    lp_ps = psB.tile([1, 1], F, tag='big')
    nc.tensor.matmul(lp_ps[:], cs['ones128'][:], acc[:], start=True, stop=True)
    loss_sb = sd.tile([1, 1], F)
    nc.scalar.copy(loss_sb[:], lp_ps[:])
    nc.sync.dma_start(dram['loss_o'][:], loss_sb[:])


def build_nc(shared, reps=1):
    nc = bacc.Bacc('TRN2', target_bir_lowering=False, debug=False,
                   num_devices=NCORE)
    dram = {}
    for name, shape, dt in SHARED_SPECS:
        shp = list(shared[name].shape) if shape is None else shape
        dram[name] = nc.dram_tensor(name, shp, dt, kind='ExternalInput').ap()
    for name, shape, dt in PER_CORE_SPECS:
        dram[name] = nc.dram_tensor(name, shape, dt, kind='ExternalInput').ap()
    dram['pred_o'] = nc.dram_tensor('pred_o', [128, BL * 4], F,
                                    kind='ExternalOutput').ap()
    dram['loss_o'] = nc.dram_tensor('loss_o', [1, 1], F,
                                    kind='ExternalOutput').ap()
    with tile.TileContext(nc) as tc:
        with ExitStack() as ctx:
            build_program(ctx, tc, dram, reps=reps)
    nc.compile()
    return nc


_CACHE = {}


def _in_maps(shared, per_core):
    maps = []
    for c in range(NCORE):
        m = {}
        for name, shape, dt in SHARED_SPECS:
            m[name] = np.ascontiguousarray(shared[name])
        for name, shape, dt in PER_CORE_SPECS:
            arr = per_core[c][name]
            m[name] = np.ascontiguousarray(arr.reshape(shape))
        maps.append(m)
    return maps


def assemble(results, t_out, qf_out):
    preds = np.zeros((B, T), np.float32)
    loss_sum = 0.0
    for c in range(NCORE):
        pt = results[c]['pred_o']
        loss_sum += float(results[c]['loss_o'][0, 0])
        for b in range(BL):
            for j in range(4):
                preds[c * BL + b, 128 * j:128 * (j + 1)] = pt[:, b * 4 + j]
    loss = np.float32(loss_sum / (B * L))
    pred = preds[:, 1:].reshape(-1).astype(np.float32)
    return loss, pred, t_out, qf_out


def kernel(**inputs):
    shared, per_core, t_out, qf_out = host_prep(inputs)
    if 'nc' not in _CACHE:
        _CACHE['nc'] = build_nc(shared)
    nc = _CACHE['nc']
    res = run_bass_kernel_spmd(nc, _in_maps(shared, per_core),
                               list(range(NCORE)))
    return assemble(res.results, t_out, qf_out)
